# revision 50
# baseline (speedup 1.0000x reference)
"""Trainium2 Bass kernel for nn_AdaptATT: grouped directional-pooling attention.

Reference computation (per fused sample s in b*groups, cg=8 channels, 128x128):
  gx           : [s, c, h, w] input slice
  sig_h/sig_w  : sigmoid(w1 @ [row-means | col-means] + b1)
  gated        : gx * sig_h * sig_w
  x1           : per-channel GroupNorm(gated) * gn_w + gn_b
  x2           : conv3x3(gx, w3) + b3
  x11          : softmax_c(mean_pix(x1)) == softmax(gn_b)   (host-known!)
  x21          : softmax_c(mean_pix(x2))
  weights      : x11 . x2 + x21 . x1   (channel contraction)
  out          : gx * sigmoid(weights)

Device strategy (per core): 2 blocks of 16 samples; partitions = (sample,
channel); free dim = flattened pixels. x11.x2 folds into a host-precomputed
1-out-channel 3x3 conv (9 block-diag matmuls on TensorE) so x2 is never
materialized; mean(x2) is reconstructed exactly from row/col sums and the 4
corner pixels. Sharding: pure data parallel over the 256 fused samples
(32 per core).
"""

import sys

if "/opt/trn_rl_repo" not in sys.path:
    sys.path.insert(0, "/opt/trn_rl_repo")

import numpy as np

B, C, H, W = 8, 256, 128, 128
GROUPS = 32
CG = C // GROUPS           # 8 channels per group
EPS = 1e-5
N_CORES = 8
BG = B * GROUPS            # 256 fused samples
S_PER_CORE = BG // N_CORES  # 32
S_BLK = 16                 # samples per device block (16*8 = 128 partitions)
N_BLK = S_PER_CORE // S_BLK  # 2
HW = H * W                 # 16384
ROWS_T = 4                 # image rows per psum tile (4*128 = 512 free)
N_TILES = H // ROWS_T      # 32 tiles per block
OUT_BATCH = 4              # tiles per output staging buffer (4*512*4B = 8KB/part)
PADW = W + 2               # padded row stride (2 left pad cols: keeps the
                           # image 4B-aligned in bf16 for DVE 2x modes)
NPIX = H * PADW + 2        # padded gx tile free size

# packed-constant layout (free-dim offsets in the [128, CPK_F] tensor)
OFF_W1T = 0            # [128, 128] block-diag w1.T / W
OFF_VALL = 128         # [128, 9*16] conv-v lhsT per tap
OFF_W3T = 272          # [128, 9*128] block-diag w3.T per tap
OFF_SBLK = 1424        # [128, 16] block-diag ones
OFF_B1T = 1440         # [128, 1]
OFF_B3T = 1441         # [128, 1]
OFF_GNW = 1442         # [128, 1]
OFF_GNB = 1443         # [128, 1]
OFF_B16 = 1444         # [16, 128] broadcast lhsT (rows 0-15)
OFF_BETA = 1572        # [16, 1]
CPK_F = 1573

# bf16 packed constants (second tensor -> own DMA lane + PE warm-up)
OFFB_VALL = 0          # [128, 9*16] conv-v lhsT per tap, bf16
OFFB_B16 = 144         # [16, 128] broadcast lhsT, bf16
CPKB_F = 272

_CACHE = {}


def _build_nc(split=True):
    import concourse.bass as bass
    import concourse.tile as tile
    from concourse import mybir

    fp32 = mybir.dt.float32
    AX = mybir.AxisListType
    OP = mybir.AluOpType
    ACT = mybir.ActivationFunctionType

    nc = bass.Bass()

    # one packed constant tensor -> ONE DMA -> one semaphore lane, so PE
    # instructions never need a second wait slot for a constant (Matmult has
    # a single HW sync-wait slot).
    bf16 = mybir.dt.bfloat16
    xb_d = nc.declare_dram_parameter("xb", [C, NPIX], bf16, isOutput=False)
    cpk_d = nc.declare_dram_parameter("cpk", [128, CPK_F], fp32, isOutput=False)
    cpkb_d = nc.declare_dram_parameter("cpkb", [128, CPKB_F], bf16,
                                       isOutput=False)
    out_d = nc.declare_dram_parameter("out", [C, HW], bf16, isOutput=True)

    from concourse.tile import add_dep_helper

    with tile.TileContext(nc) as tc:
        with (
            tc.tile_pool(name="singles", bufs=1) as singles,
            tc.tile_pool(name="gxbp", bufs=2) as gxbp,
            tc.tile_pool(name="gatedp", bufs=2) as gatedp,
            tc.tile_pool(name="small", bufs=2) as small,
            tc.tile_pool(name="x1sp", bufs=3) as x1sp,
            tc.tile_pool(name="sigp", bufs=3) as sigp,
            tc.tile_pool(name="p3sp", bufs=6) as p3sp,
            tc.tile_pool(name="outp", bufs=2) as outp,
            tc.tile_pool(name="ps_hw", bufs=1, space="PSUM") as ps_hw,
            tc.tile_pool(name="ps_a", bufs=3, space="PSUM") as ps_a,
            tc.tile_pool(name="ps_b", bufs=2, space="PSUM") as ps_b,
            tc.tile_pool(name="ps_tiny", bufs=1, space="PSUM") as ps_tiny,
        ):
            # ---- load all constants in one DMA ----
            cpk = singles.tile([128, CPK_F], fp32)
            nc.sync.dma_start(cpk[:], cpk_d[:])
            w1t = cpk[:, OFF_W1T:OFF_W1T + 128]
            vall = cpk[:, OFF_VALL:OFF_VALL + 9 * S_BLK]
            w3t = cpk[:, OFF_W3T:OFF_W3T + 9 * 128]
            sblk = cpk[:, OFF_SBLK:OFF_SBLK + S_BLK]
            b1t = cpk[:, OFF_B1T:OFF_B1T + 1]
            b3t = cpk[:, OFF_B3T:OFF_B3T + 1]
            gnwt = cpk[:, OFF_GNW:OFF_GNW + 1]
            gnbt = cpk[:, OFF_GNB:OFF_GNB + 1]
            b16 = cpk[0:S_BLK, OFF_B16:OFF_B16 + 128]
            betat = cpk[0:S_BLK, OFF_BETA:OFF_BETA + 1]
            cpkb = singles.tile([128, CPKB_F], bf16)
            nc.sync.dma_start(cpkb[:], cpkb_d[:])
            vallb = cpkb[:, OFFB_VALL:OFFB_VALL + 9 * S_BLK]
            b16b = cpkb[0:S_BLK, OFFB_B16:OFFB_B16 + 128]
            epst = singles.tile([128, 1], fp32)
            nc.vector.memset(epst[:], EPS)
            # Engine warm-ups: absorb the const-DMA lane tick into each
            # engine's observed clock so no later compute instruction needs a
            # 2nd HW sync-wait slot just for a constant operand.
            p_wu = ps_tiny.tile([1, 1], fp32, tag="wu")
            nc.tensor.matmul(p_wu[:], cpk[:, 0:1], cpk[:, 0:1])
            p_wub = ps_tiny.tile([1, 1], fp32, tag="wu")
            nc.tensor.matmul(p_wub[:], cpkb[:, 0:1], cpkb[:, 0:1])
            act_wu = singles.tile([128, 1], fp32)
            nc.scalar.copy(act_wu[:], cpk[:, 0:1])
            dve_wu = singles.tile([128, 1], fp32)
            nc.vector.tensor_copy(dve_wu[:], cpk[:, 0:1])

            coef_dep = None
            fin_pe_dep = None
            for blk in range(N_BLK):
                # gxb rows are padded host-side: pixel (i,j) at flat
                # i*PADW+1+j; col 0 of each padded row + the final element are
                # zeros, so a +-1 col shift in a conv tap reads zeros at image
                # edges. Host-cast bf16: halves input traffic, FWL-speed PE.
                gxb = gxbp.tile([128, NPIX], bf16)
                gxba = gxb[:]
                gxbr = gxb[:, 0:H * PADW].rearrange("p (h q) -> p h q", h=H)
                gxb3 = gxbr[:, :, 2:PADW]
                nc.gpsimd.dma_start(gxb[:], xb_d[blk * 128:(blk + 1) * 128, :])

                def gviewb(ir0, nr, b):
                    return bass.AP(
                        tensor=gxba.tensor,
                        offset=gxba.offset + ir0 * PADW + 2 + b,
                        ap=[[gxba.ap[0][0], 128], [PADW, nr], [1, W]])

                # ---- directional pooling: row sums (over w) and col sums (over h)
                pooled = small.tile([128, 2 * H], fp32, tag="pooled")
                i_rs = nc.vector.reduce_sum(pooled[:, 0:H], gxbr, axis=AX.X)
                i_cs = nc.vector.reduce_sum(pooled[:, H:2 * H],
                                            gxb3.transpose([0, 2, 1]),
                                            axis=AX.X)
                if coef_dep is not None:
                    # ordering-only: keep this block's (DVE-heavy) stats from
                    # preempting the previous block's stats chain -- it then
                    # fills DVE idle time during the previous final phase.
                    add_dep_helper(i_rs.ins, coef_dep, sync=False,
                                   reason="cross-block DVE ordering")
                    add_dep_helper(i_cs.ins, coef_dep, sync=False,
                                   reason="cross-block DVE ordering")

                # ---- 1x1 channel mix (w1/128 folded) + sigmoid
                p_hw = ps_hw.tile([128, 2 * H], fp32)
                i_phw = nc.tensor.matmul(p_hw[:], w1t, pooled[:])
                if fin_pe_dep is not None:
                    # ordering-only: keep this block's stats matmuls behind the
                    # previous block's final-phase matmuls in PE's in-order
                    # stream (otherwise PE stalls on this block's pooling).
                    add_dep_helper(i_phw.ins, fin_pe_dep, sync=False,
                                   reason="cross-block PE ordering")
                sig_hw = small.tile([128, 2 * H], bf16, tag="sighw")
                nc.scalar.activation(sig_hw[:], p_hw[:], ACT.Sigmoid, bias=b1t)

                # ---- gating: gated = gx * sig_h (bcast over w) * sig_w (bcast over h)
                gated = gatedp.tile([128, HW], bf16)
                g3 = gated[:].rearrange("p (h w) -> p h w", h=H)
                sh = sig_hw[:, 0:H].unsqueeze(2).to_broadcast([128, H, W])
                sw = sig_hw[:, H:2 * H].unsqueeze(1).to_broadcast([128, H, W])
                nc.vector.tensor_mul(g3, gxb3, sw)
                nc.vector.tensor_mul(g3, g3, sh)

                # ---- GroupNorm stats over pixels (per partition = per channel)
                stats = small.tile([128, 32, 6], fp32, tag="stats")
                gsub = gated[:].rearrange("p (n f) -> p n f", f=512)
                for i in range(32):
                    nc.vector.bn_stats(stats[:, i, :], gsub[:, i, :])
                mv = small.tile([128, 2], fp32, tag="mv")
                nc.vector.bn_aggr(mv[:], stats[:])
                sd = small.tile([128, 1], fp32, tag="sd")
                nc.scalar.activation(sd[:], mv[:, 1:2], ACT.Sqrt, bias=epst[:])
                rstd = small.tile([128, 1], fp32, tag="rstd")
                nc.vector.reciprocal(rstd[:], sd[:])
                # x1 = gated * (rstd*gn_w) + (gn_b - mu*rstd*gn_w)
                scale_gn = small.tile([128, 1], fp32, tag="scale_gn")
                nc.vector.tensor_mul(scale_gn[:], rstd[:], gnwt)
                mus = small.tile([128, 1], fp32, tag="mus")
                nc.vector.tensor_mul(mus[:], mv[:, 0:1], scale_gn[:])
                bias_gn = small.tile([128, 1], fp32, tag="bias_gn")
                nc.vector.tensor_sub(bias_gn[:], gnbt, mus[:])

                # ---- exact mean(x2) from row/col sums + corners
                # Tap a=dh excludes input row {a=-1: 127, a=+1: 0}; same for b/cols.
                # Row sums: pooled[:, r]; col sums: pooled[:, H+c].
                S_tot = small.tile([128, 1], fp32, tag="S_tot")
                nc.vector.reduce_sum(S_tot[:], pooled[:, 0:H], axis=AX.X)
                corners = small.tile([128, 2, 2], fp32, tag="corners")
                # corners[ta, tb] = gx[row(ta), col(tb)] with ta=0 -> row 127,
                # ta=1 -> row 0 (matches T9's step-2 tap view order a=-1,+1)
                for ta, r in ((0, H - 1), (1, 0)):
                    for tb, cc in ((0, W - 1), (1, 0)):
                        nc.vector.tensor_copy(corners[:, ta, tb:tb + 1],
                                              gxb3[:, r, cc:cc + 1])
                t3a = small.tile([128, 3], fp32, tag="t3a")
                nc.vector.tensor_sub(t3a[:, 0:1], S_tot[:], pooled[:, H - 1:H])
                nc.vector.tensor_copy(t3a[:, 1:2], S_tot[:])
                nc.vector.tensor_sub(t3a[:, 2:3], S_tot[:], pooled[:, 0:1])
                c3 = small.tile([128, 3], fp32, tag="c3")
                nc.vector.tensor_copy(c3[:, 0:1], pooled[:, 2 * H - 1:2 * H])
                nc.vector.memset(c3[:, 1:2], 0.0)
                nc.vector.tensor_copy(c3[:, 2:3], pooled[:, H:H + 1])
                T9 = small.tile([128, 3, 3], fp32, tag="T9")
                nc.vector.tensor_sub(
                    T9[:], t3a[:].unsqueeze(2).to_broadcast([128, 3, 3]),
                    c3[:].unsqueeze(1).to_broadcast([128, 3, 3]))
                corn_view = T9[:, 0:3:2, 0:3:2]
                nc.vector.tensor_add(corn_view, corn_view, corners[:])

                p_m2 = ps_tiny.tile([128, 1], fp32, tag="tiny")
                for ab in range(9):
                    nc.tensor.matmul(p_m2[:], w3t[:, ab * 128:(ab + 1) * 128],
                                     T9[:].rearrange("p a b -> p (a b)")[:, ab:ab + 1],
                                     start=(ab == 0), stop=(ab == 8))
                e8 = small.tile([128, 1], fp32, tag="e8")
                nc.scalar.activation(e8[:], p_m2[:], ACT.Exp,
                                     bias=b3t, scale=1.0 / HW)
                p_gs = ps_tiny.tile([S_BLK, 1], fp32, tag="tiny")
                nc.tensor.matmul(p_gs[:], sblk, e8[:])
                r16 = small.tile([S_BLK, 1], fp32, tag="r16")
                nc.vector.reciprocal(r16[:], p_gs[:])
                p_rb = ps_tiny.tile([128, 1], fp32, tag="tiny")
                nc.tensor.matmul(p_rb[:], b16, r16[:])
                rbs = small.tile([128, 1], fp32, tag="rbs")
                nc.scalar.copy(rbs[:], p_rb[:])
                x21c = small.tile([128, 1], fp32, tag="x21c")
                nc.vector.tensor_mul(x21c[:], e8[:], rbs[:])
                coefm = small.tile([128, S_BLK], bf16, tag="coefm")
                i_cm = nc.vector.tensor_mul(
                    coefm[:], x21c[:].to_broadcast([128, S_BLK]), sblk)
                coef_dep = i_cm.ins
                # PE warm-up on coefm's DVE tick: the first x21 matmul of the
                # block then only needs its ACT (x1s) wait slot.
                p_wu2 = ps_tiny.tile([S_BLK, 1], fp32, tag="wu")
                nc.tensor.matmul(p_wu2[:], coefm[:], coefm[:, 0:1])

                # ---- final streaming phase over 4-row tiles
                for tb in range(N_TILES // OUT_BATCH):
                    ostage = outp.tile([128, OUT_BATCH * ROWS_T * W], bf16)
                    for ti in range(OUT_BATCH):
                        t = tb * OUT_BATCH + ti
                        r0 = t * ROWS_T
                        x1s = x1sp.tile([128, ROWS_T * W], bf16)
                        # on ACT: keeps DVE clear during the final phase so the
                        # next block's stats overlap it
                        nc.scalar.activation(
                            x1s[:], gated[:, r0 * W:(r0 + ROWS_T) * W],
                            ACT.Identity, bias=bias_gn[:], scale=scale_gn[:])
                        p2 = ps_a.tile([S_BLK, ROWS_T * W], fp32)
                        # center tap first: full coverage, starts accumulation
                        nc.tensor.matmul(p2[:], vallb[:, 4 * S_BLK:5 * S_BLK],
                                         gviewb(r0, ROWS_T, 0),
                                         start=True, stop=False)
                        for ab in range(9):
                            if ab == 4:
                                continue
                            a, b = ab // 3 - 1, ab % 3 - 1
                            ir0 = max(0, r0 + a)
                            ir1 = min(H, r0 + ROWS_T + a)
                            or0, or1 = ir0 - a, ir1 - a
                            nc.tensor.matmul(
                                p2[:, (or0 - r0) * W:(or1 - r0) * W],
                                vallb[:, ab * S_BLK:(ab + 1) * S_BLK],
                                gviewb(ir0, ir1 - ir0, b),
                                start=False, stop=False)
                        nc.tensor.matmul(p2[:], coefm[:], x1s[:],
                                         start=False, stop=True)
                        sig = sigp.tile([S_BLK, ROWS_T * W], bf16)
                        nc.scalar.activation(sig[:], p2[:], ACT.Sigmoid,
                                             bias=betat)
                        p3 = ps_b.tile([128, ROWS_T * W], fp32)
                        i_p3 = nc.tensor.matmul(p3[:], b16b, sig[:])
                        fin_pe_dep = i_p3.ins
                        # evict via ACT: the p3 matmul's bank release then
                        # rides the same ACT sem as its sig wait (1 foreign
                        # wait per Matmult). The p3s slot WAR vs the DVE
                        # final-multiply is absorbed by the per-group ACT
                        # observer below.
                        p3s = p3sp.tile([128, ROWS_T * W], bf16)
                        nc.scalar.copy(p3s[:], p3[:])
                        oseg = ostage[:, ti * ROWS_T * W:(ti + 1) * ROWS_T * W]
                        nc.vector.tensor_mul(
                            oseg.rearrange("p (r c) -> p r c", r=ROWS_T),
                            gviewb(r0, ROWS_T, 0),
                            p3s[:].rearrange("p (r c) -> p r c", r=ROWS_T))
                    seg = OUT_BATCH * ROWS_T * W
                    # SWDGE: exempt from the tiny HWDGE sync-wait slot budget
                    nc.gpsimd.dma_start(
                        out_d[blk * 128:(blk + 1) * 128,
                              tb * seg:(tb + 1) * seg], ostage[:])

    if split:
        _split_multi_waits(nc, mybir)
    return nc


# TPB compute instructions have a single HW sync-wait slot on this
# toolchain ("Too many sync wait commands" at walrus codegen otherwise).
# DMAs (queue descriptors) and drains handle multiple waits fine.
_NO_SPLIT = {
    "InstEventSemaphore", "InstCall",
    "InstRegisterMove", "InstUnconditionalBranch", "InstTriggeredCopy",
}


def _split_multi_waits(nc, mybir):
    """Move all but one sync-wait of each compute instruction onto
    freshly inserted same-engine ENGINE_NOPs directly before it."""
    n = [0]

    def make_nop(engine, wait):
        n[0] += 1
        nop = mybir.InstNoOp(name=f"WSPLIT-{n[0]}", ins=[], outs=[],
                             engine=engine)
        nop.sync_info = mybir.SyncInfo(on_wait=[wait], on_update=[])
        return nop

    for bb in nc.m.functions[0].blocks:
        out = []
        for ins in bb.instructions:
            si = ins.sync_info
            waits = list(si.on_wait) if si is not None and si.on_wait else []
            if len(waits) > 1 and type(ins).__name__ not in _NO_SPLIT:
                for w in waits[:-1]:
                    out.append(make_nop(ins.engine, w))
                ins.sync_info = mybir.SyncInfo(on_wait=[waits[-1]],
                                               on_update=list(si.on_update))
            out.append(ins)
        bb.instructions[:] = out


def _host_constants(w1, b1, w3, b3, gn_w, gn_b):
    w1 = np.asarray(w1, np.float32)
    b1 = np.asarray(b1, np.float32)
    w3 = np.asarray(w3, np.float32)
    b3 = np.asarray(b3, np.float32)
    gn_w = np.asarray(gn_w, np.float32)
    gn_b = np.asarray(gn_b, np.float32)

    s = S_BLK
    cpk = np.zeros((128, CPK_F), np.float32)

    # block-diag w1^T / W : lhsT[s*8+i, s*8+o] = w1[o, i] / 128
    for k in range(s):
        cpk[k * CG:(k + 1) * CG,
            OFF_W1T + k * CG:OFF_W1T + (k + 1) * CG] = w1.T / float(W)
    cpk[:, OFF_B1T] = np.tile(b1, s)

    # x11 = softmax(gn_b) (exact: x1 spatial mean == gn_b)
    eb = np.exp(gn_b - gn_b.max())
    x11 = (eb / eb.sum()).astype(np.float32)
    cpk[0:s, OFF_BETA] = float(np.dot(x11, b3))

    # v[c, a, b] = sum_o x11[o] * w3[o, c, a, b]; lhsT[s*8+c, s] = v[c, a, b]
    v = np.einsum("o,ocab->cab", x11, w3).astype(np.float32)
    for ab in range(9):
        a, b = ab // 3, ab % 3
        for k in range(s):
            cpk[k * CG:(k + 1) * CG, OFF_VALL + ab * s + k] = v[:, a, b]

    # w3 block-diag per tap: lhsT[s*8+c, s*8+o] = w3[o, c, a, b]
    for ab in range(9):
        a, b = ab // 3, ab % 3
        for k in range(s):
            cpk[k * CG:(k + 1) * CG,
                OFF_W3T + ab * 128 + k * CG:
                OFF_W3T + ab * 128 + (k + 1) * CG] = w3[:, :, a, b].T
    cpk[:, OFF_B3T] = np.tile(b3, s)

    for k in range(s):
        cpk[k * CG:(k + 1) * CG, OFF_SBLK + k] = 1.0          # sblk
        cpk[k, OFF_B16 + k * CG:OFF_B16 + (k + 1) * CG] = 1.0  # b16

    cpk[:, OFF_GNW] = np.tile(gn_w, s)
    cpk[:, OFF_GNB] = np.tile(gn_b, s)

    import ml_dtypes
    cpkb = np.zeros((128, CPKB_F), ml_dtypes.bfloat16)
    cpkb[:, OFFB_VALL:OFFB_VALL + 9 * s] = cpk[:, OFF_VALL:OFF_VALL + 9 * s]
    cpkb[:, OFFB_B16:OFFB_B16 + 128] = cpk[:, OFF_B16:OFF_B16 + 128]
    return dict(cpk=cpk, cpkb=cpkb)


def _pad_shard(rows, dtype=np.float32):
    """[C, HW] -> [C, NPIX] with each W-row left-shifted by the shared pad col."""
    out = np.zeros((C, NPIX), dtype)
    out[:, :H * PADW].reshape(C, H, PADW)[:, :, 2:] = rows.reshape(C, H, W)
    return out


def kernel(x, w1, b1, w3, b3, gn_w, gn_b):
    from concourse.bass_utils import run_bass_kernel_spmd

    if "nc" not in _CACHE:
        _CACHE["nc"] = _build_nc()
    nc = _CACHE["nc"]

    consts = _host_constants(w1, b1, w3, b3, gn_w, gn_b)
    xv = np.asarray(x, np.float32).reshape(BG, CG, HW)
    in_maps = []
    import ml_dtypes
    for k in range(N_CORES):
        rows = xv[k * S_PER_CORE:(k + 1) * S_PER_CORE].reshape(C, HW)
        m = {"xb": _pad_shard(rows, ml_dtypes.bfloat16)}
        m.update(consts)
        in_maps.append(m)

    res = run_bass_kernel_spmd(nc, in_maps, core_ids=list(range(N_CORES)))
    outs = [np.asarray(res.results[k]["out"], np.float32)
            .reshape(S_PER_CORE, CG, H, W) for k in range(N_CORES)]
    return np.concatenate(outs, axis=0).reshape(B, C, H, W)


# revision 52
# speedup vs baseline: 1.1391x; 1.1391x over previous
"""Trainium2 Bass kernel for nn_AdaptATT: grouped directional-pooling attention.

Reference computation (per fused sample s in b*groups, cg=8 channels, 128x128):
  gx           : [s, c, h, w] input slice
  sig_h/sig_w  : sigmoid(w1 @ [row-means | col-means] + b1)
  gated        : gx * sig_h * sig_w
  x1           : per-channel GroupNorm(gated) * gn_w + gn_b
  x2           : conv3x3(gx, w3) + b3
  x11          : softmax_c(mean_pix(x1)) == softmax(gn_b)   (host-known!)
  x21          : softmax_c(mean_pix(x2))
  weights      : x11 . x2 + x21 . x1   (channel contraction)
  out          : gx * sigmoid(weights)

Device strategy (per core): 2 blocks of 16 samples; partitions = (sample,
channel); free dim = flattened pixels (rows padded to stride 130 with shared
zero pad cols so conv taps read zeros at edges and the image stays 4B-aligned
for DVE 2x modes). x11.x2 folds into a host-precomputed 1-out-channel 3x3
conv (9 block-diag bf16 matmuls on TensorE) so x2 is never materialized;
mean(x2) is reconstructed exactly from row/col sums and the 4 corner pixels.
Inputs arrive host-cast to bf16; the final multiply/output run in bf16 (host
casts back). Sharding: pure data parallel over the 256 fused samples (32 per
core, no collectives).

Toolchain quirks handled here: every TPB compute instruction gets at most
ONE sync-wait (walrus "Too many sync wait commands" otherwise) via packed
constants, engine warm-ups, careful engine assignment, and a post-schedule
pass that spills extra waits onto InstNoOps. GpSimd is DMA-issue only
(its elementwise ops hard-crash the device). A sync=False cross-block dep
keeps block N+1's stats from preempting block N's stats chain on DVE.
"""

import sys

if "/opt/trn_rl_repo" not in sys.path:
    sys.path.insert(0, "/opt/trn_rl_repo")

import numpy as np

B, C, H, W = 8, 256, 128, 128
GROUPS = 32
CG = C // GROUPS           # 8 channels per group
EPS = 1e-5
N_CORES = 8
BG = B * GROUPS            # 256 fused samples
S_PER_CORE = BG // N_CORES  # 32
S_BLK = 16                 # samples per device block (16*8 = 128 partitions)
N_BLK = S_PER_CORE // S_BLK  # 2
HW = H * W                 # 16384
ROWS_T = 4                 # image rows per psum tile (4*128 = 512 free)
N_TILES = H // ROWS_T      # 32 tiles per block
OUT_BATCH = 4              # tiles per output staging buffer (4*512*4B = 8KB/part)
PADW = W + 2               # padded row stride (2 left pad cols: keeps the
                           # image 4B-aligned in bf16 for DVE 2x modes)
NPIX = H * PADW + 2        # padded gx tile free size

# packed-constant layout (free-dim offsets in the [128, CPK_F] tensor)
OFF_W1T = 0            # [128, 128] block-diag w1.T / W
OFF_VALL = 128         # [128, 9*16] conv-v lhsT per tap
OFF_W3T = 272          # [128, 9*128] block-diag w3.T per tap
OFF_SBLK = 1424        # [128, 16] block-diag ones
OFF_B1T = 1440         # [128, 1]
OFF_B3T = 1441         # [128, 1]
OFF_GNW = 1442         # [128, 1]
OFF_GNB = 1443         # [128, 1]
OFF_B16 = 1444         # [16, 128] broadcast lhsT (rows 0-15)
OFF_BETA = 1572        # [16, 1]
CPK_F = 1573

# bf16 packed constants (second tensor -> own DMA lane + PE warm-up)
OFFB_VALL = 0          # [128, 9*16] conv-v lhsT per tap, bf16
OFFB_B16 = 144         # [16, 128] broadcast lhsT, bf16
CPKB_F = 272

_CACHE = {}


def _build_nc(split=True):
    import concourse.bass as bass
    import concourse.tile as tile
    from concourse import mybir

    fp32 = mybir.dt.float32
    AX = mybir.AxisListType
    OP = mybir.AluOpType
    ACT = mybir.ActivationFunctionType

    nc = bass.Bass()

    # one packed constant tensor -> ONE DMA -> one semaphore lane, so PE
    # instructions never need a second wait slot for a constant (Matmult has
    # a single HW sync-wait slot).
    bf16 = mybir.dt.bfloat16
    xb_d = nc.declare_dram_parameter("xb", [C, NPIX], bf16, isOutput=False)
    cpk_d = nc.declare_dram_parameter("cpk", [128, CPK_F], fp32, isOutput=False)
    cpkb_d = nc.declare_dram_parameter("cpkb", [128, CPKB_F], bf16,
                                       isOutput=False)
    out_d = nc.declare_dram_parameter("out", [C, HW], bf16, isOutput=True)

    from concourse.tile import add_dep_helper

    with tile.TileContext(nc) as tc:
        with (
            tc.tile_pool(name="singles", bufs=1) as singles,
            tc.tile_pool(name="gxbp", bufs=2) as gxbp,
            tc.tile_pool(name="gatedp", bufs=2) as gatedp,
            tc.tile_pool(name="small", bufs=2) as small,
            tc.tile_pool(name="x1sp", bufs=3) as x1sp,
            tc.tile_pool(name="sigp", bufs=3) as sigp,
            tc.tile_pool(name="p3sp", bufs=6) as p3sp,
            tc.tile_pool(name="outp", bufs=2) as outp,
            tc.tile_pool(name="ps_hw", bufs=1, space="PSUM") as ps_hw,
            tc.tile_pool(name="ps_a", bufs=3, space="PSUM") as ps_a,
            tc.tile_pool(name="ps_b", bufs=2, space="PSUM") as ps_b,
            tc.tile_pool(name="ps_tiny", bufs=1, space="PSUM") as ps_tiny,
        ):
            # ---- load all constants in one DMA ----
            cpk = singles.tile([128, CPK_F], fp32)
            nc.sync.dma_start(cpk[:], cpk_d[:])
            w1t = cpk[:, OFF_W1T:OFF_W1T + 128]
            vall = cpk[:, OFF_VALL:OFF_VALL + 9 * S_BLK]
            w3t = cpk[:, OFF_W3T:OFF_W3T + 9 * 128]
            sblk = cpk[:, OFF_SBLK:OFF_SBLK + S_BLK]
            b1t = cpk[:, OFF_B1T:OFF_B1T + 1]
            b3t = cpk[:, OFF_B3T:OFF_B3T + 1]
            gnwt = cpk[:, OFF_GNW:OFF_GNW + 1]
            gnbt = cpk[:, OFF_GNB:OFF_GNB + 1]
            b16 = cpk[0:S_BLK, OFF_B16:OFF_B16 + 128]
            betat = cpk[0:S_BLK, OFF_BETA:OFF_BETA + 1]
            cpkb = singles.tile([128, CPKB_F], bf16)
            nc.sync.dma_start(cpkb[:], cpkb_d[:])
            vallb = cpkb[:, OFFB_VALL:OFFB_VALL + 9 * S_BLK]
            b16b = cpkb[0:S_BLK, OFFB_B16:OFFB_B16 + 128]
            epst = singles.tile([128, 1], fp32)
            nc.vector.memset(epst[:], EPS)
            # Engine warm-ups: absorb the const-DMA lane tick into each
            # engine's observed clock so no later compute instruction needs a
            # 2nd HW sync-wait slot just for a constant operand.
            p_wu = ps_tiny.tile([1, 1], fp32, tag="wu")
            nc.tensor.matmul(p_wu[:], cpk[:, 0:1], cpk[:, 0:1])
            p_wub = ps_tiny.tile([1, 1], fp32, tag="wu")
            nc.tensor.matmul(p_wub[:], cpkb[:, 0:1], cpkb[:, 0:1])
            act_wu = singles.tile([128, 1], fp32)
            nc.scalar.copy(act_wu[:], cpk[:, 0:1])
            dve_wu = singles.tile([128, 1], fp32)
            nc.vector.tensor_copy(dve_wu[:], cpk[:, 0:1])

            coef_dep = None
            for blk in range(N_BLK):
                # gxb rows are padded host-side: pixel (i,j) at flat
                # i*PADW+1+j; col 0 of each padded row + the final element are
                # zeros, so a +-1 col shift in a conv tap reads zeros at image
                # edges. Host-cast bf16: halves input traffic, FWL-speed PE.
                gxb = gxbp.tile([128, NPIX], bf16)
                gxba = gxb[:]
                gxbr = gxb[:, 0:H * PADW].rearrange("p (h q) -> p h q", h=H)
                gxb3 = gxbr[:, :, 2:PADW]
                nc.gpsimd.dma_start(gxb[:], xb_d[blk * 128:(blk + 1) * 128, :])

                def gviewb(ir0, nr, b):
                    return bass.AP(
                        tensor=gxba.tensor,
                        offset=gxba.offset + ir0 * PADW + 2 + b,
                        ap=[[gxba.ap[0][0], 128], [PADW, nr], [1, W]])

                # ---- directional pooling: row sums (over w) and col sums (over h)
                pooled = small.tile([128, 2 * H], fp32, tag="pooled")
                i_rs = nc.vector.reduce_sum(pooled[:, 0:H], gxbr, axis=AX.X)
                i_cs = nc.vector.reduce_sum(pooled[:, H:2 * H],
                                            gxb3.transpose([0, 2, 1]),
                                            axis=AX.X)
                if coef_dep is not None:
                    # ordering-only: keep this block's (DVE-heavy) stats from
                    # preempting the previous block's stats chain -- it then
                    # fills DVE idle time during the previous final phase.
                    add_dep_helper(i_rs.ins, coef_dep, sync=False,
                                   reason="cross-block DVE ordering")
                    add_dep_helper(i_cs.ins, coef_dep, sync=False,
                                   reason="cross-block DVE ordering")

                # ---- 1x1 channel mix (w1/128 folded) + sigmoid
                p_hw = ps_hw.tile([128, 2 * H], fp32)
                nc.tensor.matmul(p_hw[:], w1t, pooled[:])
                sig_hw = small.tile([128, 2 * H], bf16, tag="sighw")
                nc.scalar.activation(sig_hw[:], p_hw[:], ACT.Sigmoid, bias=b1t)

                # ---- gating: gated = gx * sig_h (bcast over w) * sig_w (bcast over h)
                gated = gatedp.tile([128, HW], bf16)
                g3 = gated[:].rearrange("p (h w) -> p h w", h=H)
                sh = sig_hw[:, 0:H].unsqueeze(2).to_broadcast([128, H, W])
                sw = sig_hw[:, H:2 * H].unsqueeze(1).to_broadcast([128, H, W])
                nc.vector.tensor_mul(g3, gxb3, sw)
                nc.vector.tensor_mul(g3, g3, sh)

                # ---- GroupNorm stats over pixels (per partition = per channel)
                stats = small.tile([128, 32, 6], fp32, tag="stats")
                gsub = gated[:].rearrange("p (n f) -> p n f", f=512)
                for i in range(32):
                    nc.vector.bn_stats(stats[:, i, :], gsub[:, i, :])
                mv = small.tile([128, 2], fp32, tag="mv")
                nc.vector.bn_aggr(mv[:], stats[:])
                sd = small.tile([128, 1], fp32, tag="sd")
                nc.scalar.activation(sd[:], mv[:, 1:2], ACT.Sqrt, bias=epst[:])
                rstd = small.tile([128, 1], fp32, tag="rstd")
                nc.vector.reciprocal(rstd[:], sd[:])
                # x1 = gated * (rstd*gn_w) + (gn_b - mu*rstd*gn_w)
                scale_gn = small.tile([128, 1], fp32, tag="scale_gn")
                nc.vector.tensor_mul(scale_gn[:], rstd[:], gnwt)
                mus = small.tile([128, 1], fp32, tag="mus")
                nc.vector.tensor_mul(mus[:], mv[:, 0:1], scale_gn[:])
                bias_gn = small.tile([128, 1], fp32, tag="bias_gn")
                nc.vector.tensor_sub(bias_gn[:], gnbt, mus[:])

                # ---- exact mean(x2) from row/col sums + corners
                # Tap a=dh excludes input row {a=-1: 127, a=+1: 0}; same for b/cols.
                # Row sums: pooled[:, r]; col sums: pooled[:, H+c].
                S_tot = small.tile([128, 1], fp32, tag="S_tot")
                nc.vector.reduce_sum(S_tot[:], pooled[:, 0:H], axis=AX.X)
                corners = small.tile([128, 2, 2], fp32, tag="corners")
                # corners[ta, tb] = gx[row(ta), col(tb)] with ta=0 -> row 127,
                # ta=1 -> row 0 (matches T9's step-2 tap view order a=-1,+1)
                for ta, r in ((0, H - 1), (1, 0)):
                    for tb, cc in ((0, W - 1), (1, 0)):
                        nc.vector.tensor_copy(corners[:, ta, tb:tb + 1],
                                              gxb3[:, r, cc:cc + 1])
                t3a = small.tile([128, 3], fp32, tag="t3a")
                nc.vector.tensor_sub(t3a[:, 0:1], S_tot[:], pooled[:, H - 1:H])
                nc.vector.tensor_copy(t3a[:, 1:2], S_tot[:])
                nc.vector.tensor_sub(t3a[:, 2:3], S_tot[:], pooled[:, 0:1])
                c3 = small.tile([128, 3], fp32, tag="c3")
                nc.vector.tensor_copy(c3[:, 0:1], pooled[:, 2 * H - 1:2 * H])
                nc.vector.memset(c3[:, 1:2], 0.0)
                nc.vector.tensor_copy(c3[:, 2:3], pooled[:, H:H + 1])
                T9 = small.tile([128, 3, 3], fp32, tag="T9")
                nc.vector.tensor_sub(
                    T9[:], t3a[:].unsqueeze(2).to_broadcast([128, 3, 3]),
                    c3[:].unsqueeze(1).to_broadcast([128, 3, 3]))
                corn_view = T9[:, 0:3:2, 0:3:2]
                nc.vector.tensor_add(corn_view, corn_view, corners[:])

                p_m2 = ps_tiny.tile([128, 1], fp32, tag="tiny")
                for ab in range(9):
                    nc.tensor.matmul(p_m2[:], w3t[:, ab * 128:(ab + 1) * 128],
                                     T9[:].rearrange("p a b -> p (a b)")[:, ab:ab + 1],
                                     start=(ab == 0), stop=(ab == 8))
                e8 = small.tile([128, 1], fp32, tag="e8")
                nc.scalar.activation(e8[:], p_m2[:], ACT.Exp,
                                     bias=b3t, scale=1.0 / HW)
                p_gs = ps_tiny.tile([S_BLK, 1], fp32, tag="tiny")
                nc.tensor.matmul(p_gs[:], sblk, e8[:])
                r16 = small.tile([S_BLK, 1], fp32, tag="r16")
                nc.vector.reciprocal(r16[:], p_gs[:])
                p_rb = ps_tiny.tile([128, 1], fp32, tag="tiny")
                nc.tensor.matmul(p_rb[:], b16, r16[:])
                rbs = small.tile([128, 1], fp32, tag="rbs")
                nc.scalar.copy(rbs[:], p_rb[:])
                x21c = small.tile([128, 1], fp32, tag="x21c")
                nc.vector.tensor_mul(x21c[:], e8[:], rbs[:])
                coefm = small.tile([128, S_BLK], bf16, tag="coefm")
                i_cm = nc.vector.tensor_mul(
                    coefm[:], x21c[:].to_broadcast([128, S_BLK]), sblk)
                coef_dep = i_cm.ins
                # PE warm-up on coefm's DVE tick: the first x21 matmul of the
                # block then only needs its ACT (x1s) wait slot.
                p_wu2 = ps_tiny.tile([S_BLK, 1], fp32, tag="wu")
                nc.tensor.matmul(p_wu2[:], coefm[:], coefm[:, 0:1])

                # ---- final streaming phase over 4-row tiles
                for tb in range(N_TILES // OUT_BATCH):
                    ostage = outp.tile([128, OUT_BATCH * ROWS_T * W], bf16)
                    for ti in range(OUT_BATCH):
                        t = tb * OUT_BATCH + ti
                        r0 = t * ROWS_T
                        x1s = x1sp.tile([128, ROWS_T * W], bf16)
                        # on ACT: keeps DVE clear during the final phase so the
                        # next block's stats overlap it
                        nc.scalar.activation(
                            x1s[:], gated[:, r0 * W:(r0 + ROWS_T) * W],
                            ACT.Identity, bias=bias_gn[:], scale=scale_gn[:])
                        p2 = ps_a.tile([S_BLK, ROWS_T * W], fp32)
                        # center tap first: full coverage, starts accumulation
                        nc.tensor.matmul(p2[:], vallb[:, 4 * S_BLK:5 * S_BLK],
                                         gviewb(r0, ROWS_T, 0),
                                         start=True, stop=False)
                        for ab in range(9):
                            if ab == 4:
                                continue
                            a, b = ab // 3 - 1, ab % 3 - 1
                            ir0 = max(0, r0 + a)
                            ir1 = min(H, r0 + ROWS_T + a)
                            or0, or1 = ir0 - a, ir1 - a
                            nc.tensor.matmul(
                                p2[:, (or0 - r0) * W:(or1 - r0) * W],
                                vallb[:, ab * S_BLK:(ab + 1) * S_BLK],
                                gviewb(ir0, ir1 - ir0, b),
                                start=False, stop=False)
                        nc.tensor.matmul(p2[:], coefm[:], x1s[:],
                                         start=False, stop=True)
                        sig = sigp.tile([S_BLK, ROWS_T * W], bf16)
                        nc.scalar.activation(sig[:], p2[:], ACT.Sigmoid,
                                             bias=betat)
                        p3 = ps_b.tile([128, ROWS_T * W], fp32)
                        nc.tensor.matmul(p3[:], b16b, sig[:])
                        # evict via ACT: the p3 matmul's bank release then
                        # rides the same ACT sem as its sig wait (1 foreign
                        # wait per Matmult). The p3s slot WAR vs the DVE
                        # final-multiply is absorbed by the per-group ACT
                        # observer below.
                        p3s = p3sp.tile([128, ROWS_T * W], bf16)
                        nc.scalar.copy(p3s[:], p3[:])
                        oseg = ostage[:, ti * ROWS_T * W:(ti + 1) * ROWS_T * W]
                        nc.vector.tensor_mul(
                            oseg.rearrange("p (r c) -> p r c", r=ROWS_T),
                            gviewb(r0, ROWS_T, 0),
                            p3s[:].rearrange("p (r c) -> p r c", r=ROWS_T))
                    seg = OUT_BATCH * ROWS_T * W
                    # SWDGE: exempt from the tiny HWDGE sync-wait slot budget
                    nc.gpsimd.dma_start(
                        out_d[blk * 128:(blk + 1) * 128,
                              tb * seg:(tb + 1) * seg], ostage[:])

    if split:
        _split_multi_waits(nc, mybir)
    return nc


# TPB compute instructions have a single HW sync-wait slot on this
# toolchain ("Too many sync wait commands" at walrus codegen otherwise).
# DMAs (queue descriptors) and drains handle multiple waits fine.
_NO_SPLIT = {
    "InstEventSemaphore", "InstCall",
    "InstRegisterMove", "InstUnconditionalBranch", "InstTriggeredCopy",
}


def _split_multi_waits(nc, mybir):
    """Move all but one sync-wait of each compute instruction onto
    freshly inserted same-engine ENGINE_NOPs directly before it."""
    n = [0]

    def make_nop(engine, wait):
        n[0] += 1
        nop = mybir.InstNoOp(name=f"WSPLIT-{n[0]}", ins=[], outs=[],
                             engine=engine)
        nop.sync_info = mybir.SyncInfo(on_wait=[wait], on_update=[])
        return nop

    for bb in nc.m.functions[0].blocks:
        out = []
        for ins in bb.instructions:
            si = ins.sync_info
            waits = list(si.on_wait) if si is not None and si.on_wait else []
            if len(waits) > 1 and type(ins).__name__ not in _NO_SPLIT:
                for w in waits[:-1]:
                    out.append(make_nop(ins.engine, w))
                ins.sync_info = mybir.SyncInfo(on_wait=[waits[-1]],
                                               on_update=list(si.on_update))
            out.append(ins)
        bb.instructions[:] = out


def _host_constants(w1, b1, w3, b3, gn_w, gn_b):
    w1 = np.asarray(w1, np.float32)
    b1 = np.asarray(b1, np.float32)
    w3 = np.asarray(w3, np.float32)
    b3 = np.asarray(b3, np.float32)
    gn_w = np.asarray(gn_w, np.float32)
    gn_b = np.asarray(gn_b, np.float32)

    s = S_BLK
    cpk = np.zeros((128, CPK_F), np.float32)

    # block-diag w1^T / W : lhsT[s*8+i, s*8+o] = w1[o, i] / 128
    for k in range(s):
        cpk[k * CG:(k + 1) * CG,
            OFF_W1T + k * CG:OFF_W1T + (k + 1) * CG] = w1.T / float(W)
    cpk[:, OFF_B1T] = np.tile(b1, s)

    # x11 = softmax(gn_b) (exact: x1 spatial mean == gn_b)
    eb = np.exp(gn_b - gn_b.max())
    x11 = (eb / eb.sum()).astype(np.float32)
    cpk[0:s, OFF_BETA] = float(np.dot(x11, b3))

    # v[c, a, b] = sum_o x11[o] * w3[o, c, a, b]; lhsT[s*8+c, s] = v[c, a, b]
    v = np.einsum("o,ocab->cab", x11, w3).astype(np.float32)
    for ab in range(9):
        a, b = ab // 3, ab % 3
        for k in range(s):
            cpk[k * CG:(k + 1) * CG, OFF_VALL + ab * s + k] = v[:, a, b]

    # w3 block-diag per tap: lhsT[s*8+c, s*8+o] = w3[o, c, a, b]
    for ab in range(9):
        a, b = ab // 3, ab % 3
        for k in range(s):
            cpk[k * CG:(k + 1) * CG,
                OFF_W3T + ab * 128 + k * CG:
                OFF_W3T + ab * 128 + (k + 1) * CG] = w3[:, :, a, b].T
    cpk[:, OFF_B3T] = np.tile(b3, s)

    for k in range(s):
        cpk[k * CG:(k + 1) * CG, OFF_SBLK + k] = 1.0          # sblk
        cpk[k, OFF_B16 + k * CG:OFF_B16 + (k + 1) * CG] = 1.0  # b16

    cpk[:, OFF_GNW] = np.tile(gn_w, s)
    cpk[:, OFF_GNB] = np.tile(gn_b, s)

    import ml_dtypes
    cpkb = np.zeros((128, CPKB_F), ml_dtypes.bfloat16)
    cpkb[:, OFFB_VALL:OFFB_VALL + 9 * s] = cpk[:, OFF_VALL:OFF_VALL + 9 * s]
    cpkb[:, OFFB_B16:OFFB_B16 + 128] = cpk[:, OFF_B16:OFF_B16 + 128]
    return dict(cpk=cpk, cpkb=cpkb)


def _pad_shard(rows, dtype=np.float32):
    """[C, HW] -> [C, NPIX] with each W-row left-shifted by the shared pad col."""
    out = np.zeros((C, NPIX), dtype)
    out[:, :H * PADW].reshape(C, H, PADW)[:, :, 2:] = rows.reshape(C, H, W)
    return out


def kernel(x, w1, b1, w3, b3, gn_w, gn_b):
    from concourse.bass_utils import run_bass_kernel_spmd

    if "nc" not in _CACHE:
        _CACHE["nc"] = _build_nc()
    nc = _CACHE["nc"]

    consts = _host_constants(w1, b1, w3, b3, gn_w, gn_b)
    xv = np.asarray(x, np.float32).reshape(BG, CG, HW)
    in_maps = []
    import ml_dtypes
    for k in range(N_CORES):
        rows = xv[k * S_PER_CORE:(k + 1) * S_PER_CORE].reshape(C, HW)
        m = {"xb": _pad_shard(rows, ml_dtypes.bfloat16)}
        m.update(consts)
        in_maps.append(m)

    res = run_bass_kernel_spmd(nc, in_maps, core_ids=list(range(N_CORES)))
    outs = [np.asarray(res.results[k]["out"], np.float32)
            .reshape(S_PER_CORE, CG, H, W) for k in range(N_CORES)]
    return np.concatenate(outs, axis=0).reshape(B, C, H, W)
